# revision 6
# baseline (speedup 1.0000x reference)
"""Multi-head causal attention (B=4,S=2048,D=1024,H=16,d=64) on 8 trn2 cores.

Sharding: 8 cores = 4 batches x 2 head-groups.  Core c = 2b+g handles batch b
and heads 8g..8g+7 for ALL 2048 queries (full causal).  This removes the K/V
projection redundancy of a (batch x seq-half) split: each core projects
Q, K, V once for its own 8 heads only.

Out-projection is feature-sharded: each core contracts its 512 features of
Wo over all 2048 tokens, producing a PARTIAL y^T [1024, 2048].  The two
partials of a pair are summed during unshard on the host (out = (p0+p1).T
+ bo).  No collectives; the device program is identical on every core and
even the mask constant is shared (plain triangular diagonal block).

On-device layout tricks (kept from the seq-split baseline):
  - scores computed transposed, S^T[sk, sq]; softmax denominators come free
    by appending a ones-column to V (row 64 of the AV psum accumulator).
  - exp on ScalarE with the 1/sqrt(64) folded into its free affine scale.
  - denominator broadcast across partitions via a bf16 ones-row matmul
    (the fp32 version cost 4x on the PE).
  - all matmul operands bf16 (full PE rate), fp32 PSUM accumulation.
"""

import numpy as np
import ml_dtypes

B, S, D = 4, 2048, 1024
H, DH = 16, 64
HL = 8                # heads per core
NHP = HL // 2         # 4 local head pairs
NKT = S // 128        # 16 k-tiles
NDC = D // 128        # 8 contraction chunks
SQH = S // 2          # 1024, query-half length
VST = 66              # V column stride per head (64 V cols + 1 ones + 1 pad)

BF16 = ml_dtypes.bfloat16

_cache = {}


def _build():
    import concourse.bass as bass
    import concourse.tile as tile
    import concourse.mybir as mybir
    from concourse import bacc
    from contextlib import ExitStack

    dt = mybir.dt
    AF = mybir.ActivationFunctionType

    nc = bacc.Bacc(
        "TRN2",
        target_bir_lowering=False,
        debug=False,
        enable_asserts=False,
        num_devices=8,
    )

    qt_d = nc.dram_tensor("qT", [D, S], dt.bfloat16, kind="ExternalInput").ap()
    kt_d = nc.dram_tensor("kT", [D, S], dt.bfloat16, kind="ExternalInput").ap()
    vt_d = nc.dram_tensor("vT", [D, S], dt.bfloat16, kind="ExternalInput").ap()
    wq_d = nc.dram_tensor("Wq", [NHP, 128, NDC, 2, DH], dt.bfloat16, kind="ExternalInput").ap()
    wk_d = nc.dram_tensor("Wk", [NHP, 128, NDC, 2, DH], dt.bfloat16, kind="ExternalInput").ap()
    wv_d = nc.dram_tensor("Wv", [128, NDC, HL, DH], dt.bfloat16, kind="ExternalInput").ap()
    wot_d = nc.dram_tensor("WoT4", [NHP, 128, D], dt.bfloat16, kind="ExternalInput").ap()
    mask_d = nc.dram_tensor("mask", [128, 128], dt.bfloat16, kind="ExternalInput").ap()
    y_d = nc.dram_tensor("yT", [D, S], dt.bfloat16, kind="ExternalOutput").ap()

    with tile.TileContext(nc) as tc, ExitStack() as ctx:
        const = ctx.enter_context(tc.tile_pool(name="const", bufs=1))
        work = ctx.enter_context(tc.tile_pool(name="work", bufs=3))
        pp = ctx.enter_context(tc.tile_pool(name="pp", bufs=2, space="PSUM"))

        # ---- constants (tiles declared here; DMAs issued after the Q
        # projection weights so the critical-path loads go first) ----------
        mask_sb = const.tile([128, 128], dt.bfloat16, tag="mask", name="mask")
        wv_sb = const.tile([128, NDC, HL, DH], dt.bfloat16, tag="wv", name="wv")
        wot_sb = [
            const.tile([128, D], dt.bfloat16, tag=f"wot{f}", name=f"wot{f}")
            for f in range(NHP)
        ]

        def load_consts():
            # bulk constants go on the Activation HWDGE queue so they don't
            # head-of-line-block the critical q/k slab loads on the SP queue
            nc.scalar.dma_start(out=wv_sb, in_=wv_d)
            nc.scalar.dma_start(out=mask_sb, in_=mask_d)
            for f in range(NHP):
                nc.scalar.dma_start(out=wot_sb[f], in_=wot_d[f])

        # persistent projected tensors
        qt_sb = [
            const.tile([128, S], dt.bfloat16, tag=f"qt{hp}", name=f"qt{hp}")
            for hp in range(NHP)
        ]
        kt_sb = [
            const.tile([128, S], dt.bfloat16, tag=f"kt{hp}", name=f"kt{hp}")
            for hp in range(NHP)
        ]
        v_sb = [
            const.tile([128, HL, VST], dt.bfloat16, tag=f"v{t}", name=f"v{t}")
            for t in range(NKT)
        ]
        ot_sb = [
            const.tile([128, S], dt.bfloat16, tag=f"ot{hp}", name=f"ot{hp}")
            for hp in range(NHP)
        ]

        for t in range(NKT):
            # ones column per head for the softmax denominators
            nc.vector.memset(v_sb[t][:, :, 64:65], 1.0)

        ones_bf = const.tile([1, 64], dt.bfloat16, tag="ones_bf")
        nc.vector.memset(ones_bf, 1.0)

        # ---- phase 1: projections -------------------------------------
        def load_dmajor_batch(src_d, c0, width, tag="tt", split=1, bufs=2, eng=None):
            """Load a D-major [D, c0:c0+width] slab into
            tt[128 D-part, dc, width] bf16.  split>1 breaks the DMA into
            dc-chunks so the first matmul can start sooner."""
            eng = eng or nc.sync
            tt = work.tile([128, NDC, 512], dt.bfloat16, tag=tag, bufs=bufs)
            src = src_d[:, c0 : c0 + width].rearrange("(dc p) c -> p dc c", p=128)
            step = NDC // split
            for i in range(0, NDC, step):
                eng.dma_start(
                    out=tt[:, i : i + step, 0:width], in_=src[:, i : i + step]
                )
            return tt

        # slab prefetch bookkeeping: issue DMAs in an order that matches
        # first-use times on the (FIFO) DMA queue
        slabs = {}

        def prefetch(nm, src_d, bi, split=1, bufs=2, eng=None):
            slabs[(nm, bi)] = load_dmajor_batch(
                src_d, 512 * bi, 512, tag="tt" + nm, split=split, bufs=bufs, eng=eng
            )

        def load_w(wpool, nm, wsrc):
            w_sb = []
            for hp in range(NHP):
                t = wpool.tile(
                    [128, NDC, 2, DH], dt.bfloat16,
                    tag=f"{nm}{hp}", name=f"{nm}{hp}",
                )
                nc.sync.dma_start(out=t, in_=wsrc[hp])
                w_sb.append(t)
            return w_sb

        with tc.tile_pool(name="wqk", bufs=1) as wpool:
            # q slab 0 rides the Activation queue in parallel with the wq
            # weight loads on the SP queue; the rest follow on SP (bufs=4)
            prefetch("q", qt_d, 0, split=4, bufs=4, eng=nc.scalar)
            wq_sb = load_w(wpool, "wq", wq_d)
            for bi in range(1, 4):
                prefetch("q", qt_d, bi, bufs=4)
            wk_sb = load_w(wpool, "wk", wk_d)
            prefetch("k", kt_d, 0)
            prefetch("k", kt_d, 1)
            load_consts()  # wv first inside; wot behind (Activation queue)
            prefetch("v", vt_d, 0, eng=nc.scalar)
            prefetch("v", vt_d, 1, eng=nc.scalar)

            # Q^T and K^T projections: [2-head dq partitions, s]
            for nm, src_d, w_sb, out_sb in (
                ("q", qt_d, wq_sb, qt_sb),
                ("k", kt_d, wk_sb, kt_sb),
            ):
                for bi in range(4):
                    tt = slabs.pop((nm, bi))
                    for hp in range(NHP):
                        ps = pp.tile([128, 512], dt.float32, tag="acc")
                        for dc in range(NDC):
                            nc.tensor.matmul(
                                ps,
                                lhsT=w_sb[hp][:, dc],
                                rhs=tt[:, dc, :],
                                start=(dc == 0),
                                stop=(dc == NDC - 1),
                            )
                        nc.vector.tensor_copy(
                            out=out_sb[hp][:, 512 * bi : 512 * (bi + 1)], in_=ps
                        )
                    # refill this slab slot only after its readers are issued
                    if nm == "k" and bi + 2 < 4:
                        prefetch(nm, src_d, bi + 2)

        # V projection is issued one k-tile group at a time, interleaved
        # with the first head-pair's score/exp stream (phase 2) so the
        # Activation engine's exp pipeline starts ~27us earlier.
        vstate = {"next": 0}

        def vproj_group():
            kt = vstate["next"]
            if kt >= NKT:
                return
            vstate["next"] = kt + 1
            bi, tsub = kt // 4, kt % 4
            tt = slabs[("v", bi)]
            ps = pp.tile([128, 512], dt.float32, tag="acc")
            for dc in range(NDC):
                nc.tensor.matmul(
                    ps,
                    lhsT=tt[:, dc, 128 * tsub : 128 * (tsub + 1)],
                    rhs=wv_sb[:, dc].rearrange("p h v -> p (h v)"),
                    start=(dc == 0),
                    stop=(dc == NDC - 1),
                )
            nc.vector.tensor_copy(
                out=v_sb[kt][:, :, 0:DH],
                in_=ps.rearrange("p (h v) -> p h v", v=DH),
            )
            if tsub == 3:
                del slabs[("v", bi)]
                if bi + 2 < 4:
                    # SP queue: it is idle during attention, while the
                    # Activation queue is the exp-stream pacer here
                    prefetch("v", vt_d, bi + 2)

        # ---- phase 2: attention ---------------------------------------
        # Per head pair, queries processed in two halves of 1024.  For a
        # query half, k-tile t covers keys [128t, 128(t+1)); valid query
        # columns are the local suffix [qs, 1024) with qs as below; tiles
        # whose 128x128 diagonal block intersects get the triangular mask.
        def tiles_of(qh):
            # (t, qs, diag) for this query half
            out = []
            for t in range(8 * (qh + 1)):
                qs = max(0, 128 * (t - 8 * qh))
                out.append((t, qs, t >= 8 * qh))
            return out

        def epilogue_dens(st):
            # DVE den copies issue first so they overlap the next block's
            # score matmuls; the PE broadcast then finds them ready.
            avs, _, _ = st
            dens = []
            for s in range(2):
                den = work.tile([1, SQH], dt.bfloat16, tag="den", bufs=2)
                if s == 0:
                    # ScalarE is free right after the last (small) exp; DVE
                    # still drains the diagonal mask mul -- run one den copy
                    # on each so both broadcasts find their operand sooner
                    nc.scalar.activation(
                        out=den, in_=avs[s][64:65, :], func=AF.Copy
                    )
                else:
                    nc.vector.tensor_copy(out=den, in_=avs[s][64:65, :])
                dens.append(den)
            return dens

        def epilogue_finish(st, dens):
            avs, hp, qh = st
            qoff = SQH * qh
            for s in range(2):
                av = avs[s]
                for ch in range(2):
                    nc.tensor.matmul(
                        av[64:128, 512 * ch : 512 * (ch + 1)],
                        lhsT=ones_bf,
                        rhs=dens[s][:, 512 * ch : 512 * (ch + 1)],
                        start=True,
                        stop=True,
                    )
            for s in range(2):
                po = 64 * s
                av = avs[s]
                rb = work.tile([64, SQH], dt.bfloat16, tag="rb", bufs=2)
                with nc.allow_low_precision(reason="1/den at bf16; den itself is bf16"):
                    nc.vector.reciprocal(out=rb, in_=av[64:128, :])
                nc.vector.tensor_mul(
                    ot_sb[hp][po : po + 64, qoff : qoff + SQH], av[0:64, :], rb
                )

        pend = None
        for hp in range(NHP):
            for qh in range(2):
                qoff = SQH * qh
                tl = tiles_of(qh)
                t_half = max(t for t, qs, _ in tl if qs < 512)
                t_last = tl[-1][0]
                # During (hp0, qh0) the AV matmuls are deferred: the acc psum
                # slots host the interleaved V-projection groups instead, and
                # the exp'd score tiles wait in the deep pt ring.
                defer_av = hp == 0 and qh == 0
                avs = None if defer_av else [
                    pp.tile([128, SQH], dt.float32, tag="acc", name=f"av{s}")
                    for s in range(2)
                ]

                def issue_scores(t, qs, diag):
                    L = SQH - qs
                    pts = []
                    scs = [
                        pp.tile([128, SQH], dt.float32, tag="sc", name=f"sc{s}")
                        for s in range(2)
                    ]
                    for s in range(2):
                        po = 64 * s
                        lhsT = kt_sb[hp][po : po + 64, 128 * t : 128 * (t + 1)]
                        for c0 in range(0, L, 512):
                            c1 = min(c0 + 512, L)
                            nc.tensor.matmul(
                                scs[s][:, c0:c1],
                                lhsT=lhsT,
                                rhs=qt_sb[hp][po : po + 64, qoff + qs + c0 : qoff + qs + c1],
                                start=True,
                                stop=True,
                            )
                    for s in range(2):
                        pt = work.tile([128, SQH], dt.bfloat16, tag="pt", bufs=16,
                                       name=f"pt{s}")
                        nc.scalar.activation(
                            out=pt[:, :L], in_=scs[s][:, :L], func=AF.Exp, scale=0.125
                        )
                        if diag:
                            nc.vector.tensor_mul(pt[:, 0:128], pt[:, 0:128], mask_sb)
                        pts.append(pt)
                    return pts

                def issue_av(avs, t, qs, pts):
                    L = SQH - qs
                    for s in range(2):
                        h = 2 * hp + s
                        av = avs[s]
                        vh = v_sb[t][:, h, 0:65]
                        if qs < 512:
                            nc.tensor.matmul(
                                av[0:65, qs:512], lhsT=vh,
                                rhs=pts[s][:, 0 : 512 - qs],
                                start=(t == 0), stop=(t == t_half),
                            )
                            nc.tensor.matmul(
                                av[0:65, 512:SQH], lhsT=vh,
                                rhs=pts[s][:, 512 - qs : L],
                                start=(t == 0), stop=(t == t_last),
                            )
                        else:
                            nc.tensor.matmul(
                                av[0:65, qs:SQH], lhsT=vh, rhs=pts[s][:, 0:L],
                                start=False, stop=(t == t_last),
                            )

                prev = None
                stored = []
                for i, (t, qs, diag) in enumerate(tl):
                    if i == 0 and pend is not None:
                        dens = epilogue_dens(pend)
                    pts = issue_scores(t, qs, diag)
                    if i == 0 and pend is not None:
                        epilogue_finish(pend, dens)
                        pend = None
                    if defer_av:
                        vproj_group()
                        vproj_group()
                        stored.append((t, qs, pts))
                    else:
                        if prev is not None:
                            issue_av(avs, *prev)
                        prev = (t, qs, pts)
                if defer_av:
                    avs = [
                        pp.tile([128, SQH], dt.float32, tag="acc", name=f"av{s}")
                        for s in range(2)
                    ]
                    for st_t, st_qs, st_pts in stored:
                        issue_av(avs, st_t, st_qs, st_pts)
                else:
                    issue_av(avs, *prev)
                pend = (avs, hp, qh)

        # ---- phase 3: output projection (partial y over local features) --
        # 512-token psum groups, alternating tags so 4 slots rotate and the
        # psum->sbuf copies + output DMAs pipeline behind the matmuls.
        # The first two groups (sc slots, qh0 columns -- ready long ago) are
        # issued BEFORE the final epilogue: they keep the PE busy while the
        # last den copies run, and the epilogue's broadcasts then start with
        # their operands already in SBUF.
        for tq in range(4):
            for dc in range(NDC):
                if tq == 0 and dc == 2:
                    dens = epilogue_dens(pend)
                    epilogue_finish(pend, dens)
                    pend = None
                tag = "sc" if (tq, dc) in ((0, 0), (0, 1)) else (
                    "acc" if dc % 2 else "sc"
                )
                yp = pp.tile([128, 512], dt.float32, tag=tag)
                for f in range(NHP):
                    nc.tensor.matmul(
                        yp,
                        lhsT=wot_sb[f][:, 128 * dc : 128 * (dc + 1)],
                        rhs=ot_sb[f][:, 512 * tq : 512 * (tq + 1)],
                        start=(f == 0),
                        stop=(f == NHP - 1),
                    )
                ys = work.tile([128, 512], dt.bfloat16, tag="ys", bufs=4)
                nc.vector.tensor_copy(out=ys, in_=yp)
                nc.sync.dma_start(
                    out=y_d[128 * dc : 128 * (dc + 1), 512 * tq : 512 * (tq + 1)],
                    in_=ys,
                )

    nc.compile()
    return nc


def _get_program():
    if "nc" not in _cache:
        _cache["nc"] = _build()
    return _cache["nc"]


def _make_in_maps(q, k, v, Wq, Wk, Wv, Wo, bo):
    q = np.asarray(q, np.float32)
    k = np.asarray(k, np.float32)
    v = np.asarray(v, np.float32)

    def _pack_qk(W):
        # [HL, D, DH] -> [NHP, 128, NDC, 2, DH]: partition-major so the
        # device load is one fully contiguous DMA per head pair
        W = np.asarray(W, np.float32).astype(BF16)
        return np.ascontiguousarray(
            W.reshape(NHP, 2, NDC, 128, DH).transpose(0, 3, 2, 1, 4)
        )

    WoT = np.ascontiguousarray(np.asarray(Wo, np.float32).T)  # [feat, dout]
    mask = np.triu(np.ones((128, 128), np.float32)).astype(BF16)

    gw = []
    for g in range(2):
        hs = slice(HL * g, HL * (g + 1))
        gw.append(
            {
                "Wq": _pack_qk(np.asarray(Wq, np.float32)[hs]),
                "Wk": _pack_qk(np.asarray(Wk, np.float32)[hs]),
                "Wv": np.ascontiguousarray(
                    np.asarray(Wv, np.float32)[hs].astype(BF16)
                    .transpose(1, 0, 2).reshape(NDC, 128, HL, DH)
                    .transpose(1, 0, 2, 3)
                ),
                "WoT4": np.ascontiguousarray(
                    WoT[512 * g : 512 * (g + 1)].reshape(NHP, 128, D)
                ).astype(BF16),
            }
        )

    qT = [np.ascontiguousarray(q[b].T).astype(BF16) for b in range(B)]
    kT = [np.ascontiguousarray(k[b].T).astype(BF16) for b in range(B)]
    vT = [np.ascontiguousarray(v[b].T).astype(BF16) for b in range(B)]
    in_maps = []
    for c in range(8):
        b, g = c // 2, c % 2
        in_maps.append(
            {
                "qT": qT[b],
                "kT": kT[b],
                "vT": vT[b],
                "mask": mask,
                **gw[g],
            }
        )
    return in_maps


def _assemble(bo, per_core_yT):
    """Sum pair partials and unshard: out[b] = (yT[2b] + yT[2b+1]).T + bo."""
    bo = np.asarray(bo, np.float32)
    out = np.empty((B, S, D), np.float32)
    for b in range(B):
        acc = per_core_yT[2 * b].astype(np.float32) + per_core_yT[2 * b + 1].astype(
            np.float32
        )
        out[b] = acc.T + bo
    return out


def kernel(q, k, v, Wq, Wk, Wv, Wo, bo, trace=False):
    from concourse.bass_utils import run_bass_kernel_spmd

    nc = _get_program()
    in_maps = _make_in_maps(q, k, v, Wq, Wk, Wv, Wo, bo)
    res = run_bass_kernel_spmd(nc, in_maps, core_ids=list(range(8)), trace=trace)
    _cache["last_results"] = res
    return _assemble(bo, [res.results[c]["yT"] for c in range(8)])


def last_exec_time_ns():
    res = _cache.get("last_results")
    return getattr(res, "exec_time_ns", None) if res is not None else None


def benchmark(q, k, v, Wq, Wk, Wv, Wo, bo, iters=20):
    """Steady-state device timing: jit once, keep inputs device-resident,
    time repeated executions.  Returns (per_iter_seconds_list, output)."""
    import time
    import jax
    import jax.numpy as jnp
    import concourse.mybir as mybir
    from jax.experimental.shard_map import shard_map
    from jax.sharding import Mesh, NamedSharding, PartitionSpec
    from concourse import bass2jax

    nc = _get_program()
    bass2jax.install_neuronx_cc_hook()

    in_maps = _make_in_maps(q, k, v, Wq, Wk, Wv, Wo, bo)

    partition_name = nc.partition_id_tensor.name if nc.partition_id_tensor else None
    in_names, out_names, out_avals, zero_shapes = [], [], [], []
    for alloc in nc.m.functions[0].allocations:
        if not isinstance(alloc, mybir.MemoryLocationSet):
            continue
        name = alloc.memorylocations[0].name
        if alloc.kind == "ExternalInput":
            if name != partition_name:
                in_names.append(name)
        elif alloc.kind == "ExternalOutput":
            out_names.append(name)
            shape = tuple(alloc.tensor_shape)
            dtype = mybir.dt.np(alloc.dtype)
            out_avals.append(jax.core.ShapedArray(shape, dtype))
            zero_shapes.append((shape, dtype))
    n_params = len(in_names)
    all_names = in_names + out_names
    if partition_name is not None:
        all_names.append(partition_name)
    donate = tuple(range(n_params, n_params + len(out_names)))

    n_outs = len(out_names)

    def _one(args):
        operands = list(args)
        if partition_name is not None:
            operands.append(bass2jax.partition_id_tensor())
        outs = bass2jax._bass_exec_p.bind(
            *operands,
            out_avals=tuple(out_avals),
            in_names=tuple(all_names),
            out_names=tuple(out_names),
            lowering_input_output_aliases=(),
            sim_require_finite=True,
            sim_require_nnan=True,
            nc=nc,
        )
        return tuple(outs)

    def _body(*args):
        return _one(args)

    devices = jax.devices()[:8]
    mesh = Mesh(np.asarray(devices), ("core",))
    spec = PartitionSpec("core")
    sh = NamedSharding(mesh, spec)
    f1 = jax.jit(
        shard_map(
            _body, mesh=mesh,
            in_specs=(spec,) * (n_params + n_outs),
            out_specs=(spec,) * n_outs,
            check_rep=False,
        ),
        donate_argnums=donate,
        keep_unused=True,
    )
    concat_in = [
        jax.device_put(
            np.concatenate([np.asarray(in_maps[c][nm]) for c in range(8)], axis=0), sh
        )
        for nm in in_names
    ]

    zfns = [
        jax.jit(
            (lambda s, d: (lambda: jnp.zeros((8 * s[0], *s[1:]), d)))(s, d),
            out_shardings=sh,
        )
        for s, d in zero_shapes
    ]

    def make_zeros(n):
        return [[zf() for zf in zfns] for _ in range(n)]

    # warmup (compile)
    out_arrs = f1(*concat_in, *make_zeros(1)[0])
    jax.block_until_ready(out_arrs)

    # slope fit across chain depths, robust to bimodal dispatch latency
    depths = [4, 16, 40]
    samples = {d: [] for d in depths}
    for _ in range(iters):
        for d in depths:
            zsl = make_zeros(d)
            jax.block_until_ready(zsl)
            t0 = time.perf_counter()
            outs = [f1(*concat_in, *zsl[i]) for i in range(d)]
            jax.block_until_ready(outs)
            samples[d].append(time.perf_counter() - t0)
            out_arrs = outs[-1]
    mins = {d: min(v) for d, v in samples.items()}
    slopes = [
        (mins[d2] - mins[d1]) / (d2 - d1)
        for d1, d2 in zip(depths, depths[1:])
        if mins[d2] > mins[d1]
    ]
    per_exec = float(min(slopes)) if slopes else float("nan")
    _cache["bench"] = {
        "mins": mins,
        "per_exec": per_exec,
    }

    yT_all = np.asarray(out_arrs[out_names.index("yT")]).reshape(8, D, S)
    out = _assemble(bo, [yT_all[c] for c in range(8)])
    return samples[depths[0]], out


# revision 7
# speedup vs baseline: 1.6036x; 1.6036x over previous
"""Multi-head causal attention (B=4,S=2048,D=1024,H=16,d=64) on 8 trn2 cores.

Sharding: 8 cores = 4 batches x 2 head-groups.  Core c = 2b+g handles batch b
and heads 8g..8g+7 for ALL 2048 queries (full causal).  This removes the K/V
projection redundancy of a (batch x seq-half) split: each core projects
Q, K, V once for its own 8 heads only.

Out-projection is feature-sharded: each core contracts its 512 features of
Wo over all 2048 tokens, producing a PARTIAL y^T [1024, 2048].  The two
partials of a pair are summed during unshard on the host (out = (p0+p1).T
+ bo).  No collectives; the device program is identical on every core and
even the mask constant is shared (plain triangular diagonal block).

On-device layout tricks (kept from the seq-split baseline):
  - scores computed transposed, S^T[sk, sq]; softmax denominators come free
    by appending a ones-column to V (row 64 of the AV psum accumulator).
  - exp on ScalarE with the 1/sqrt(64) folded into its free affine scale.
  - denominator broadcast across partitions via a bf16 ones-row matmul
    (the fp32 version cost 4x on the PE).
  - all matmul operands bf16 (full PE rate), fp32 PSUM accumulation.
"""

import numpy as np
import ml_dtypes

B, S, D = 4, 2048, 1024
H, DH = 16, 64
HL = 8                # heads per core
NHP = HL // 2         # 4 local head pairs
NKT = S // 128        # 16 k-tiles
NDC = D // 128        # 8 contraction chunks
SQH = S // 2          # 1024, query-half length
VST = 66              # V column stride per head (64 V cols + 1 ones + 1 pad)

BF16 = ml_dtypes.bfloat16

_cache = {}


def _build():
    import concourse.bass as bass
    import concourse.tile as tile
    import concourse.mybir as mybir
    from concourse import bacc
    from contextlib import ExitStack

    dt = mybir.dt
    AF = mybir.ActivationFunctionType

    nc = bacc.Bacc(
        "TRN2",
        target_bir_lowering=False,
        debug=False,
        enable_asserts=False,
        num_devices=8,
    )

    qt_d = nc.dram_tensor("qT", [D, S], dt.bfloat16, kind="ExternalInput").ap()
    kt_d = nc.dram_tensor("kT", [D, S], dt.bfloat16, kind="ExternalInput").ap()
    vt_d = nc.dram_tensor("vT", [D, S], dt.bfloat16, kind="ExternalInput").ap()
    wq_d = nc.dram_tensor("Wq", [NHP, 128, NDC, 2, DH], dt.bfloat16, kind="ExternalInput").ap()
    wk_d = nc.dram_tensor("Wk", [NHP, 128, NDC, 2, DH], dt.bfloat16, kind="ExternalInput").ap()
    wv_d = nc.dram_tensor("Wv", [128, NDC, HL, DH], dt.bfloat16, kind="ExternalInput").ap()
    wot_d = nc.dram_tensor("WoT4", [NHP, 128, D], dt.bfloat16, kind="ExternalInput").ap()
    mask_d = nc.dram_tensor("mask", [128, 128], dt.bfloat16, kind="ExternalInput").ap()
    y_d = nc.dram_tensor("yT", [D, S], dt.bfloat16, kind="ExternalOutput").ap()

    with tile.TileContext(nc) as tc, ExitStack() as ctx:
        const = ctx.enter_context(tc.tile_pool(name="const", bufs=1))
        work = ctx.enter_context(tc.tile_pool(name="work", bufs=3))
        pp = ctx.enter_context(tc.tile_pool(name="pp", bufs=2, space="PSUM"))

        # ---- constants (tiles declared here; DMAs issued after the Q
        # projection weights so the critical-path loads go first) ----------
        mask_sb = const.tile([128, 128], dt.bfloat16, tag="mask", name="mask")
        wv_sb = const.tile([128, NDC, HL, DH], dt.bfloat16, tag="wv", name="wv")
        wot_sb = [
            const.tile([128, D], dt.bfloat16, tag=f"wot{f}", name=f"wot{f}")
            for f in range(NHP)
        ]

        def load_consts():
            # bulk constants go on the Activation HWDGE queue so they don't
            # head-of-line-block the critical q/k slab loads on the SP queue
            nc.scalar.dma_start(out=wv_sb, in_=wv_d)
            nc.scalar.dma_start(out=mask_sb, in_=mask_d)
            for f in range(NHP):
                nc.scalar.dma_start(out=wot_sb[f], in_=wot_d[f])

        # persistent projected tensors
        qt_sb = [
            const.tile([128, S], dt.bfloat16, tag=f"qt{hp}", name=f"qt{hp}")
            for hp in range(NHP)
        ]
        kt_sb = [
            const.tile([128, S], dt.bfloat16, tag=f"kt{hp}", name=f"kt{hp}")
            for hp in range(NHP)
        ]
        v_sb = [
            const.tile([128, HL, VST], dt.bfloat16, tag=f"v{t}", name=f"v{t}")
            for t in range(NKT)
        ]
        ot_sb = [
            const.tile([128, S], dt.bfloat16, tag=f"ot{hp}", name=f"ot{hp}")
            for hp in range(NHP)
        ]

        for t in range(NKT):
            # ones column per head for the softmax denominators
            nc.vector.memset(v_sb[t][:, :, 64:65], 1.0)

        ones_bf = const.tile([1, 64], dt.bfloat16, tag="ones_bf")
        nc.vector.memset(ones_bf, 1.0)

        # ---- phase 1: projections -------------------------------------
        def load_dmajor_batch(src_d, c0, width, tag="tt", split=1, bufs=2, eng=None):
            """Load a D-major [D, c0:c0+width] slab into
            tt[128 D-part, dc, width] bf16.  split>1 breaks the DMA into
            dc-chunks so the first matmul can start sooner."""
            eng = eng or nc.sync
            tt = work.tile([128, NDC, 512], dt.bfloat16, tag=tag, bufs=bufs)
            src = src_d[:, c0 : c0 + width].rearrange("(dc p) c -> p dc c", p=128)
            step = NDC // split
            for i in range(0, NDC, step):
                eng.dma_start(
                    out=tt[:, i : i + step, 0:width], in_=src[:, i : i + step]
                )
            return tt

        # slab prefetch bookkeeping: issue DMAs in an order that matches
        # first-use times on the (FIFO) DMA queue
        slabs = {}

        def prefetch(nm, src_d, bi, split=1, bufs=2, eng=None):
            slabs[(nm, bi)] = load_dmajor_batch(
                src_d, 512 * bi, 512, tag="tt" + nm, split=split, bufs=bufs, eng=eng
            )

        def load_w(wpool, nm, wsrc):
            w_sb = []
            for hp in range(NHP):
                t = wpool.tile(
                    [128, NDC, 2, DH], dt.bfloat16,
                    tag=f"{nm}{hp}", name=f"{nm}{hp}",
                )
                nc.sync.dma_start(out=t, in_=wsrc[hp])
                w_sb.append(t)
            return w_sb

        with tc.tile_pool(name="wqk", bufs=1) as wpool:
            # q slab 0 rides the Activation queue in parallel with the wq
            # weight loads on the SP queue; the rest follow on SP (bufs=4)
            prefetch("q", qt_d, 0, split=4, bufs=4, eng=nc.scalar)
            wq_sb = load_w(wpool, "wq", wq_d)
            for bi in range(1, 4):
                prefetch("q", qt_d, bi, bufs=4)
            wk_sb = load_w(wpool, "wk", wk_d)
            prefetch("k", kt_d, 0)
            prefetch("k", kt_d, 1)
            load_consts()  # wv first inside; wot behind (Activation queue)
            prefetch("v", vt_d, 0, eng=nc.scalar)
            prefetch("v", vt_d, 1, eng=nc.scalar)

            # Q^T and K^T projections: [2-head dq partitions, s]
            for nm, src_d, w_sb, out_sb in (
                ("q", qt_d, wq_sb, qt_sb),
                ("k", kt_d, wk_sb, kt_sb),
            ):
                for bi in range(4):
                    tt = slabs.pop((nm, bi))
                    for hp in range(NHP):
                        ps = pp.tile([128, 512], dt.float32, tag="acc")
                        for dc in range(NDC):
                            nc.tensor.matmul(
                                ps,
                                lhsT=w_sb[hp][:, dc],
                                rhs=tt[:, dc, :],
                                start=(dc == 0),
                                stop=(dc == NDC - 1),
                            )
                        nc.vector.tensor_copy(
                            out=out_sb[hp][:, 512 * bi : 512 * (bi + 1)], in_=ps
                        )
                    # refill this slab slot only after its readers are issued
                    if nm == "k" and bi + 2 < 4:
                        prefetch(nm, src_d, bi + 2)

        # V projection is issued one k-tile group at a time, interleaved
        # with the first head-pair's score/exp stream (phase 2) so the
        # Activation engine's exp pipeline starts ~27us earlier.
        vstate = {"next": 0}

        def vproj_group():
            kt = vstate["next"]
            if kt >= NKT:
                return
            vstate["next"] = kt + 1
            bi, tsub = kt // 4, kt % 4
            tt = slabs[("v", bi)]
            ps = pp.tile([128, 512], dt.float32, tag="acc")
            for dc in range(NDC):
                nc.tensor.matmul(
                    ps,
                    lhsT=tt[:, dc, 128 * tsub : 128 * (tsub + 1)],
                    rhs=wv_sb[:, dc].rearrange("p h v -> p (h v)"),
                    start=(dc == 0),
                    stop=(dc == NDC - 1),
                )
            nc.vector.tensor_copy(
                out=v_sb[kt][:, :, 0:DH],
                in_=ps.rearrange("p (h v) -> p h v", v=DH),
            )
            if tsub == 3:
                del slabs[("v", bi)]
                if bi + 2 < 4:
                    # SP queue: it is idle during attention, while the
                    # Activation queue is the exp-stream pacer here
                    prefetch("v", vt_d, bi + 2)

        # ---- phase 2: attention ---------------------------------------
        # Per head pair, queries processed in two halves of 1024.  For a
        # query half, k-tile t covers keys [128t, 128(t+1)); valid query
        # columns are the local suffix [qs, 1024) with qs as below; tiles
        # whose 128x128 diagonal block intersects get the triangular mask.
        def tiles_of(qh):
            # (t, qs, diag) for this query half
            out = []
            for t in range(8 * (qh + 1)):
                qs = max(0, 128 * (t - 8 * qh))
                out.append((t, qs, t >= 8 * qh))
            return out

        def epilogue_dens(st):
            # DVE den copies issue first so they overlap the next block's
            # score matmuls; the PE broadcast then finds them ready.
            avs, _, _ = st
            dens = []
            for s in range(2):
                den = work.tile([1, SQH], dt.bfloat16, tag="den", bufs=2)
                if s == 0:
                    # ScalarE is free right after the last (small) exp; DVE
                    # still drains the diagonal mask mul -- run one den copy
                    # on each so both broadcasts find their operand sooner
                    nc.scalar.activation(
                        out=den, in_=avs[s][64:65, :], func=AF.Copy
                    )
                else:
                    nc.vector.tensor_copy(out=den, in_=avs[s][64:65, :])
                dens.append(den)
            return dens

        def epilogue_finish(st, dens):
            avs, hp, qh = st
            qoff = SQH * qh
            for s in range(2):
                av = avs[s]
                for ch in range(2):
                    nc.tensor.matmul(
                        av[64:128, 512 * ch : 512 * (ch + 1)],
                        lhsT=ones_bf,
                        rhs=dens[s][:, 512 * ch : 512 * (ch + 1)],
                        start=True,
                        stop=True,
                    )
            for s in range(2):
                po = 64 * s
                av = avs[s]
                rb = work.tile([64, SQH], dt.bfloat16, tag="rb", bufs=2)
                with nc.allow_low_precision(reason="1/den at bf16; den itself is bf16"):
                    nc.vector.reciprocal(out=rb, in_=av[64:128, :])
                nc.vector.tensor_mul(
                    ot_sb[hp][po : po + 64, qoff : qoff + SQH], av[0:64, :], rb
                )

        pend = None
        for hp in range(NHP):
            for qh in range(2):
                qoff = SQH * qh
                tl = tiles_of(qh)
                t_half = max(t for t, qs, _ in tl if qs < 512)
                t_last = tl[-1][0]
                # During (hp0, qh0) the AV matmuls are deferred: the acc psum
                # slots host the interleaved V-projection groups instead, and
                # the exp'd score tiles wait in the deep pt ring.
                defer_av = hp == 0 and qh == 0
                avs = None if defer_av else [
                    pp.tile([128, SQH], dt.float32, tag="acc", name=f"av{s}")
                    for s in range(2)
                ]

                def issue_scores(t, qs, diag):
                    L = SQH - qs
                    pts = []
                    scs = [
                        pp.tile([128, SQH], dt.float32, tag="sc", name=f"sc{s}")
                        for s in range(2)
                    ]
                    for s in range(2):
                        po = 64 * s
                        lhsT = kt_sb[hp][po : po + 64, 128 * t : 128 * (t + 1)]
                        for c0 in range(0, L, 512):
                            c1 = min(c0 + 512, L)
                            nc.tensor.matmul(
                                scs[s][:, c0:c1],
                                lhsT=lhsT,
                                rhs=qt_sb[hp][po : po + 64, qoff + qs + c0 : qoff + qs + c1],
                                start=True,
                                stop=True,
                            )
                    for s in range(2):
                        pt = work.tile([128, SQH], dt.bfloat16, tag="pt", bufs=16,
                                       name=f"pt{s}")
                        nc.scalar.activation(
                            out=pt[:, :L], in_=scs[s][:, :L], func=AF.Exp, scale=0.125
                        )
                        if diag:
                            nc.vector.tensor_mul(pt[:, 0:128], pt[:, 0:128], mask_sb)
                        pts.append(pt)
                    return pts

                def issue_av(avs, t, qs, pts):
                    L = SQH - qs
                    for s in range(2):
                        h = 2 * hp + s
                        av = avs[s]
                        vh = v_sb[t][:, h, 0:65]
                        if qs < 512:
                            nc.tensor.matmul(
                                av[0:65, qs:512], lhsT=vh,
                                rhs=pts[s][:, 0 : 512 - qs],
                                start=(t == 0), stop=(t == t_half),
                            )
                            nc.tensor.matmul(
                                av[0:65, 512:SQH], lhsT=vh,
                                rhs=pts[s][:, 512 - qs : L],
                                start=(t == 0), stop=(t == t_last),
                            )
                        else:
                            nc.tensor.matmul(
                                av[0:65, qs:SQH], lhsT=vh, rhs=pts[s][:, 0:L],
                                start=False, stop=(t == t_last),
                            )

                prev = None
                stored = []
                for i, (t, qs, diag) in enumerate(tl):
                    if i == 0 and pend is not None:
                        dens = epilogue_dens(pend)
                    pts = issue_scores(t, qs, diag)
                    if i == 0 and pend is not None:
                        epilogue_finish(pend, dens)
                        pend = None
                    if defer_av:
                        vproj_group()
                        vproj_group()
                        stored.append((t, qs, pts))
                    else:
                        if prev is not None:
                            issue_av(avs, *prev)
                        prev = (t, qs, pts)
                if defer_av:
                    avs = [
                        pp.tile([128, SQH], dt.float32, tag="acc", name=f"av{s}")
                        for s in range(2)
                    ]
                    for st_t, st_qs, st_pts in stored:
                        issue_av(avs, st_t, st_qs, st_pts)
                else:
                    issue_av(avs, *prev)
                pend = (avs, hp, qh)

        # ---- phase 3: output projection (partial y over local features) --
        # 512-token psum groups, alternating tags so 4 slots rotate and the
        # psum->sbuf copies + output DMAs pipeline behind the matmuls.
        # The first two groups (sc slots, qh0 columns -- ready long ago) are
        # issued BEFORE the final epilogue: they keep the PE busy while the
        # last den copies run, and the epilogue's broadcasts then start with
        # their operands already in SBUF.
        for tq in range(4):
            for dc in range(NDC):
                if tq == 0 and dc == 2:
                    dens = epilogue_dens(pend)
                    epilogue_finish(pend, dens)
                    pend = None
                tag = "sc" if (tq, dc) in ((0, 0), (0, 1)) else (
                    "acc" if dc % 2 else "sc"
                )
                yp = pp.tile([128, 512], dt.float32, tag=tag)
                for f in range(NHP):
                    nc.tensor.matmul(
                        yp,
                        lhsT=wot_sb[f][:, 128 * dc : 128 * (dc + 1)],
                        rhs=ot_sb[f][:, 512 * tq : 512 * (tq + 1)],
                        start=(f == 0),
                        stop=(f == NHP - 1),
                    )
                ys = work.tile([128, 512], dt.bfloat16, tag="ys", bufs=4)
                # alternate copy engines: ScalarE is idle in phase 3, and the
                # split halves the end-of-kernel copy drain
                if dc % 2:
                    nc.scalar.activation(out=ys, in_=yp, func=AF.Copy)
                else:
                    nc.vector.tensor_copy(out=ys, in_=yp)
                nc.sync.dma_start(
                    out=y_d[128 * dc : 128 * (dc + 1), 512 * tq : 512 * (tq + 1)],
                    in_=ys,
                )

    nc.compile()
    return nc


def _get_program():
    if "nc" not in _cache:
        _cache["nc"] = _build()
    return _cache["nc"]


def _make_in_maps(q, k, v, Wq, Wk, Wv, Wo, bo):
    q = np.asarray(q, np.float32)
    k = np.asarray(k, np.float32)
    v = np.asarray(v, np.float32)

    def _pack_qk(W):
        # [HL, D, DH] -> [NHP, 128, NDC, 2, DH]: partition-major so the
        # device load is one fully contiguous DMA per head pair
        W = np.asarray(W, np.float32).astype(BF16)
        return np.ascontiguousarray(
            W.reshape(NHP, 2, NDC, 128, DH).transpose(0, 3, 2, 1, 4)
        )

    WoT = np.ascontiguousarray(np.asarray(Wo, np.float32).T)  # [feat, dout]
    mask = np.triu(np.ones((128, 128), np.float32)).astype(BF16)

    gw = []
    for g in range(2):
        hs = slice(HL * g, HL * (g + 1))
        gw.append(
            {
                "Wq": _pack_qk(np.asarray(Wq, np.float32)[hs]),
                "Wk": _pack_qk(np.asarray(Wk, np.float32)[hs]),
                "Wv": np.ascontiguousarray(
                    np.asarray(Wv, np.float32)[hs].astype(BF16)
                    .transpose(1, 0, 2).reshape(NDC, 128, HL, DH)
                    .transpose(1, 0, 2, 3)
                ),
                "WoT4": np.ascontiguousarray(
                    WoT[512 * g : 512 * (g + 1)].reshape(NHP, 128, D)
                ).astype(BF16),
            }
        )

    qT = [np.ascontiguousarray(q[b].T).astype(BF16) for b in range(B)]
    kT = [np.ascontiguousarray(k[b].T).astype(BF16) for b in range(B)]
    vT = [np.ascontiguousarray(v[b].T).astype(BF16) for b in range(B)]
    in_maps = []
    for c in range(8):
        b, g = c // 2, c % 2
        in_maps.append(
            {
                "qT": qT[b],
                "kT": kT[b],
                "vT": vT[b],
                "mask": mask,
                **gw[g],
            }
        )
    return in_maps


def _assemble(bo, per_core_yT):
    """Sum pair partials and unshard: out[b] = (yT[2b] + yT[2b+1]).T + bo."""
    bo = np.asarray(bo, np.float32)
    out = np.empty((B, S, D), np.float32)
    for b in range(B):
        acc = per_core_yT[2 * b].astype(np.float32) + per_core_yT[2 * b + 1].astype(
            np.float32
        )
        out[b] = acc.T + bo
    return out


def kernel(q, k, v, Wq, Wk, Wv, Wo, bo, trace=False):
    from concourse.bass_utils import run_bass_kernel_spmd

    nc = _get_program()
    in_maps = _make_in_maps(q, k, v, Wq, Wk, Wv, Wo, bo)
    res = run_bass_kernel_spmd(nc, in_maps, core_ids=list(range(8)), trace=trace)
    _cache["last_results"] = res
    return _assemble(bo, [res.results[c]["yT"] for c in range(8)])


def last_exec_time_ns():
    res = _cache.get("last_results")
    return getattr(res, "exec_time_ns", None) if res is not None else None


def benchmark(q, k, v, Wq, Wk, Wv, Wo, bo, iters=20):
    """Steady-state device timing: jit once, keep inputs device-resident,
    time repeated executions.  Returns (per_iter_seconds_list, output)."""
    import time
    import jax
    import jax.numpy as jnp
    import concourse.mybir as mybir
    from jax.experimental.shard_map import shard_map
    from jax.sharding import Mesh, NamedSharding, PartitionSpec
    from concourse import bass2jax

    nc = _get_program()
    bass2jax.install_neuronx_cc_hook()

    in_maps = _make_in_maps(q, k, v, Wq, Wk, Wv, Wo, bo)

    partition_name = nc.partition_id_tensor.name if nc.partition_id_tensor else None
    in_names, out_names, out_avals, zero_shapes = [], [], [], []
    for alloc in nc.m.functions[0].allocations:
        if not isinstance(alloc, mybir.MemoryLocationSet):
            continue
        name = alloc.memorylocations[0].name
        if alloc.kind == "ExternalInput":
            if name != partition_name:
                in_names.append(name)
        elif alloc.kind == "ExternalOutput":
            out_names.append(name)
            shape = tuple(alloc.tensor_shape)
            dtype = mybir.dt.np(alloc.dtype)
            out_avals.append(jax.core.ShapedArray(shape, dtype))
            zero_shapes.append((shape, dtype))
    n_params = len(in_names)
    all_names = in_names + out_names
    if partition_name is not None:
        all_names.append(partition_name)
    donate = tuple(range(n_params, n_params + len(out_names)))

    n_outs = len(out_names)

    def _one(args):
        operands = list(args)
        if partition_name is not None:
            operands.append(bass2jax.partition_id_tensor())
        outs = bass2jax._bass_exec_p.bind(
            *operands,
            out_avals=tuple(out_avals),
            in_names=tuple(all_names),
            out_names=tuple(out_names),
            lowering_input_output_aliases=(),
            sim_require_finite=True,
            sim_require_nnan=True,
            nc=nc,
        )
        return tuple(outs)

    def _body(*args):
        return _one(args)

    devices = jax.devices()[:8]
    mesh = Mesh(np.asarray(devices), ("core",))
    spec = PartitionSpec("core")
    sh = NamedSharding(mesh, spec)
    f1 = jax.jit(
        shard_map(
            _body, mesh=mesh,
            in_specs=(spec,) * (n_params + n_outs),
            out_specs=(spec,) * n_outs,
            check_rep=False,
        ),
        donate_argnums=donate,
        keep_unused=True,
    )
    concat_in = [
        jax.device_put(
            np.concatenate([np.asarray(in_maps[c][nm]) for c in range(8)], axis=0), sh
        )
        for nm in in_names
    ]

    zfns = [
        jax.jit(
            (lambda s, d: (lambda: jnp.zeros((8 * s[0], *s[1:]), d)))(s, d),
            out_shardings=sh,
        )
        for s, d in zero_shapes
    ]

    def make_zeros(n):
        return [[zf() for zf in zfns] for _ in range(n)]

    # warmup (compile)
    out_arrs = f1(*concat_in, *make_zeros(1)[0])
    jax.block_until_ready(out_arrs)

    # slope fit across chain depths, robust to bimodal dispatch latency
    depths = [4, 16, 40]
    samples = {d: [] for d in depths}
    for _ in range(iters):
        for d in depths:
            zsl = make_zeros(d)
            jax.block_until_ready(zsl)
            t0 = time.perf_counter()
            outs = [f1(*concat_in, *zsl[i]) for i in range(d)]
            jax.block_until_ready(outs)
            samples[d].append(time.perf_counter() - t0)
            out_arrs = outs[-1]
    mins = {d: min(v) for d, v in samples.items()}
    slopes = [
        (mins[d2] - mins[d1]) / (d2 - d1)
        for d1, d2 in zip(depths, depths[1:])
        if mins[d2] > mins[d1]
    ]
    per_exec = float(min(slopes)) if slopes else float("nan")
    _cache["bench"] = {
        "mins": mins,
        "per_exec": per_exec,
    }

    yT_all = np.asarray(out_arrs[out_names.index("yT")]).reshape(8, D, S)
    out = _assemble(bo, [yT_all[c] for c in range(8)])
    return samples[depths[0]], out
